# revision 16
# baseline (speedup 1.0000x reference)
"""Trainium2 Bass kernel for nn_LA_283467842715.

Math (per batch b, head h of 16, each head owning 128 contiguous channels):
  means/maxs over (128 group channels x 2x2 patch) -> [B,16,4,4]
  tiny MLP (16->1 conv, relu, 1->16 conv) on means and maxs, fused by a
  2->1 conv, bilinear-upsampled 4x4->8x8, sigmoid -> gate
  out = x * (1 + gate[b, h, y, x])

v4 (pure data parallel, 32 batches/core, 4 chunks of 8 batches):
  HBM/SBUF layout is HOST-PERMUTED patch-major bf16: partition
  p = b*16+h, free f = ij*512 + c*4 + d, with ij = i*4+j the 4x4 patch
  grid, c the in-group channel and d = dy*2+dx the 2x2 patch offset.
  Every hot access pattern is then dense:
   - ACT: per-patch SUMS are activation(Copy, accum_out) over dense
     512-elem runs (fp32 accumulate), 16/chunk + relu/sigmoid/gate+1.
   - DVE: pairwise bf16 tensor_max tree (dense 2x mode), patch-max
     reduce, two tiny PSUM->SBUF copies, and the final broadcast
     multiply out = gate1 * x (dense innermost run of 4).
   - PE: the whole gate MLP as tiny fp32 matmuls with block-diagonal
     stationaries (K2 columns pre-permuted to patch-major).
   - GpSimd: deliberately UNUSED (concurrent GpSimd + DVE fully blocks
     one of them on the shared SBUF port -- HW-verified).
   - sync: ALL DMA (HWDGE).  Every chunk's loads are issued up front;
     stores follow in FIFO order as each half-multiply lands.
  The gate/multiply/store stage is software-pipelined one chunk behind
  the stats stage so ACT's accum batches and DVE's trees run back to
  back instead of waiting on the previous chunk's MLP.
"""

import sys

if "/opt/trn_rl_repo" not in sys.path:
    sys.path.insert(0, "/opt/trn_rl_repo")

import numpy as np

HEAD = 16
B, C, H, W = 256, 2048, 8, 8
NCORES = 8
BPC = B // NCORES          # 32 batches per core
CHUNK_B = 8                # batches per SBUF chunk (8*16 heads = 128 partitions)
NCHUNK = BPC // CHUNK_B    # 4
C16 = C // HEAD            # 128 channels per head group
SPAT = H * W               # 64
FREE = C16 * SPAT          # 8192 elems per partition

LAST_EXEC_NS = None        # filled when trace=True

# Per-patch engine assignment for the 32 stat accumulations
# (16 sums, 16 maxes), each one tensor_scalar/activation op over a
# dense 512-elem run with a free-dim accumulator:
#   DVE ~0.35us each (4x mode), GpSimd ~0.65us (Q7 1-input loop),
#   ACT ~1.0us (1 elem/cycle + 280ns accumulator read; sums only).
# Short GpSimd ops bound the shared-SBUF-port stall a concurrent DVE
# op can suffer to well under a microsecond.
SUM_ENG = ["A"] * 11 + ["V"] * 5
MAX_ENG = ["V"] * 16



def _upsample_matrix():
    """U[8,4]: bilinear 4->8, half-pixel centers (align_corners=False)."""
    U = np.zeros((8, 4), dtype=np.float64)
    for y in range(8):
        src = (y + 0.5) / 2.0 - 0.5
        i0 = int(np.floor(src))
        t = src - i0
        U[y, min(max(i0, 0), 3)] += 1.0 - t
        U[y, min(max(i0 + 1, 0), 3)] += t
    return U


# pixel index s = y*8+x for patch-major column c' = ij*4 + d
_PERM = np.array([(2 * (ij // 4) + d // 2) * 8 + 2 * (ij % 4) + d % 2
                  for ij in range(16) for d in range(4)])

# consts column layout (fp32, [128, NCONST])
_W1A = 0        # [:, 0:8]    W1a[p=(b,h), b'] = (b==b') * w1[h] / 512
_W1B = 8        # [:, 8:16]   W1b[p=(b,h), b'] = (b==b') * w1[h]
_K2C = 16       # [0:32, 16:80]   K2c[(t,g), c'] = wv[t] * K2[g, s(c')]
_BRW = 80       # [0:8, 80:208]   Brw2[b', (b,h)] = (b==b') * w2[h]
_BETA = 208     # [:, 208]    (wv0+wv1)*b2[h] + bv
_B1 = 209       # [:, 209]    b1
_IDN = 210      # [:, 210:338] identity
NCONST = 338


def _pack_params(w1, b1, w2, b2, wv, bv):
    w1 = np.asarray(w1, np.float64).reshape(HEAD)
    w2 = np.asarray(w2, np.float64).reshape(HEAD)
    b2 = np.asarray(b2, np.float64).reshape(HEAD)
    wv = np.asarray(wv, np.float64).reshape(2)
    bv = float(np.asarray(bv, np.float64))
    b1 = float(np.asarray(b1, np.float64))

    U = _upsample_matrix()
    # K2[g, s] = U[y,i] * U[x,j], g = i*4+j, s = y*8+x
    K2 = np.einsum("yi,xj->ijyx", U, U).reshape(16, 64)
    K2P = K2[:, _PERM]     # columns in patch-major order

    p = np.arange(128)
    CONST = np.zeros((128, NCONST))
    for b in range(CHUNK_B):
        CONST[b * HEAD:(b + 1) * HEAD, _W1A + b] = w1 / 512.0
        CONST[b * HEAD:(b + 1) * HEAD, _W1B + b] = w1
        CONST[b, _BRW + b * HEAD:_BRW + (b + 1) * HEAD] = w2
    CONST[0:16, _K2C:_K2C + 64] = wv[0] * K2P
    CONST[16:32, _K2C:_K2C + 64] = wv[1] * K2P
    CONST[:, _BETA] = (wv[0] + wv[1]) * b2[p % 16] + bv
    CONST[:, _B1] = b1
    CONST[:, _IDN:_IDN + 128] = np.eye(128)
    return {"consts": np.ascontiguousarray(CONST, np.float32)}


def _pack_x(shard):
    """[BPC, C, H, W] (any dtype) -> [NCHUNK, 128, FREE] patch-major."""
    xs = shard.reshape(NCHUNK, CHUNK_B, HEAD, C16, 4, 2, 4, 2)
    #                   ci      b       h     c    i  dy j  dx
    xt = xs.transpose(0, 1, 2, 4, 6, 3, 5, 7)  # ci b h i j c dy dx
    return np.ascontiguousarray(xt).reshape(NCHUNK, 128, FREE)


def _unpack_out(arr):
    """[NCHUNK, 128, FREE] patch-major -> [BPC, C, H, W]."""
    xt = arr.reshape(NCHUNK, CHUNK_B, HEAD, 4, 4, C16, 2, 2)
    xs = xt.transpose(0, 1, 2, 5, 3, 6, 4, 7)  # ci b h c i dy j dx
    return xs.reshape(BPC, C, H, W)


def _split_multi_waits(nc, mybir):
    """Walrus codegen on this path only supports one sync-wait per
    instruction; hoist extras onto standalone InstEventSemaphore waits
    inserted right before, on the same engine."""
    n = 0
    for fn in nc.m.functions:
        for bb in fn.blocks:
            out = []
            for ins in bb.instructions:
                si = getattr(ins, "sync_info", None)
                waits = list(si.on_wait) if (si and si.on_wait) else []
                if len(waits) > 1:
                    for w in waits[:-1]:
                        n += 1
                        ev = mybir.InstEventSemaphore(
                            name=f"WSPLIT-{n}",
                            sync_info=mybir.SyncInfo(on_wait=[w], on_update=[]),
                        )
                        ev.engine = ins.engine
                        out.append(ev)
                    si.on_wait = [waits[-1]]
                out.append(ins)
            bb.instructions[:] = out


def _max_tree(nc, bass, X, T):
    """Pairwise max over the 128 channels of each patch-major 512-block
    of X [128, 8192] -> dense [128, 64] = [p, (ij, d)] view of T.
    8 dense bf16 tensor_tensor ops; the first two consume the two DMA
    halves (patches 0:8 / 8:16) independently."""
    op = nc.vector.tensor_max

    def ap(view, off, blk, nij):
        v = view[:, :]
        return bass.AP(tensor=v.tensor, offset=v.offset + off,
                       ap=[v.ap[0], [2 * blk, nij], [1, blk]])

    # L1 per half: [p, ij, 512] -> [p, ij, 256]
    op(T[:, 0:2048], ap(X, 0, 256, 8), ap(X, 256, 256, 8))
    op(T[:, 2048:4096], ap(X, 4096, 256, 8), ap(X, 4096 + 256, 256, 8))
    lo, blk, src = 4096, 128, 0
    while blk >= 4:
        op(T[:, lo:lo + 16 * blk], ap(T, src, blk, 16), ap(T, src + blk, blk, 16))
        src = lo
        lo += 16 * blk
        blk //= 2
    return T[:, src:src + 64]


def _build(split_waits=True):
    import concourse.bass as bass
    import concourse.tile as tile
    from concourse import mybir

    f32 = mybir.dt.float32
    bf16 = mybir.dt.bfloat16
    nc = bass.Bass()

    xd = nc.dram_tensor("x", [NCHUNK, 128, FREE], bf16, kind="ExternalInput")
    od = nc.dram_tensor("out", [NCHUNK, 128, FREE], bf16, kind="ExternalOutput")
    cd = nc.dram_tensor("consts", [128, NCONST], f32, kind="ExternalInput")

    AF = mybir.ActivationFunctionType
    HALF = FREE // 2

    with tile.TileContext(nc) as tc:
        with (
            tc.tile_pool(name="singles", bufs=1) as singles,
            tc.tile_pool(name="xin", bufs=NCHUNK) as xpool,
            tc.tile_pool(name="oout", bufs=3) as opool,
            tc.tile_pool(name="small", bufs=2) as small,
            tc.tile_pool(name="psum", bufs=2, space="PSUM") as psum,
        ):
            CN = singles.tile([128, NCONST], f32)
            nc.sync.dma_start(out=CN, in_=cd[:, :])
            w1a = CN[:, _W1A:_W1A + 8]
            w1b = CN[:, _W1B:_W1B + 8]
            k2c = CN[0:32, _K2C:_K2C + 64]
            brw = CN[0:8, _BRW:_BRW + 128]
            beta = CN[:, _BETA:_BETA + 1]
            b1c = CN[0:8, _B1:_B1 + 1]
            idn8 = CN[0:8, _IDN:_IDN + 8]

            TM = singles.tile([128, FREE], bf16)       # max-tree scratch
            DUM = singles.tile([128, 512], bf16)       # ACT accum dummy out
            DUMV = singles.tile([128, 512], bf16)      # DVE accum dummy out
            DUMG = singles.tile([128, 512], bf16)      # GpSimd accum dummy out

            # prefetch every chunk before any store can queue on the ring
            Xs = []
            for ci in range(NCHUNK):
                X = xpool.tile([128, FREE], bf16, tag="X")
                nc.sync.dma_start(out=X[:, 0:HALF], in_=xd[ci, :, 0:HALF])
                nc.sync.dma_start(out=X[:, HALF:FREE], in_=xd[ci, :, HALF:FREE])
                Xs.append(X)

            sms = []

            def stats(ci):
                X = Xs[ci]
                sm = small.tile([128, 32], f32, tag="sm")
                sms.append(sm)
                # 32 dense per-patch stat accums (sum cols 0:16, max
                # cols 16:32), split across DVE / ACT / GpSimd
                ALU = mybir.AluOpType
                for ij in range(16):
                    xij = X[:, ij * 512:(ij + 1) * 512]
                    for col, red, eng in ((ij, ALU.add, SUM_ENG[ij]),
                                          (16 + ij, ALU.max, MAX_ENG[ij])):
                        acc = sm[:, col:col + 1]
                        if eng == "V":
                            nc.vector.tensor_scalar(
                                DUMV[:, :], xij, 1.0, None,
                                ALU.mult, red, accum_out=acc)
                        else:
                            nc.scalar.activation(DUM[:, :], xij, AF.Copy,
                                                 accum_out=acc)

            def gate_mul_store(ci):
                X, sm = Xs[ci], sms[ci]
                O = opool.tile([128, FREE], bf16, tag="O")

                P1 = psum.tile([8, 32], f32, tag="P1")
                nc.tensor.matmul(P1[:, 0:16], w1a, sm[:, 0:16])
                nc.tensor.matmul(P1[:, 16:32], w1b, sm[:, 16:32])
                R = small.tile([8, 32], f32, tag="R")
                nc.scalar.activation(R[:, :], P1[:, :], AF.Relu, bias=b1c)

                RT = psum.tile([32, 8], f32, tag="RT")
                nc.tensor.transpose(RT[:, :], R[:, :], idn8)
                RTs = small.tile([32, 8], f32, tag="RTs")
                nc.scalar.copy(RTs[:, :], RT[:, :])

                QP = psum.tile([8, 64], f32, tag="QP")
                nc.tensor.matmul(QP[:, :], RTs[:, :], k2c)
                qup = small.tile([8, 64], f32, tag="qup")
                nc.scalar.copy(qup[:, :], QP[:, :])

                Z = psum.tile([128, 64], f32, tag="Z")
                nc.tensor.matmul(Z[:, :], brw, qup[:, :])
                gate = small.tile([128, 64], bf16, tag="gate")
                nc.scalar.activation(gate[:, :], Z[:, :], AF.Sigmoid, bias=beta)
                gate1 = small.tile([128, 64], bf16, tag="gate1")
                nc.vector.tensor_scalar_add(gate1[:, :], gate[:, :], 1.0)

                # out = gate1 * x (gate broadcast over c), one dense TT
                # per half so each store trails its half
                g = gate1[:, :]
                for hi, lo in enumerate((0, HALF)):
                    Xh = X[:, lo:lo + HALF].rearrange(
                        "p (ij c d) -> p ij c d", ij=8, c=C16)
                    Oh = O[:, lo:lo + HALF].rearrange(
                        "p (ij c d) -> p ij c d", ij=8, c=C16)
                    gb = bass.AP(tensor=g.tensor, offset=g.offset + hi * 32,
                                 ap=[g.ap[0], [4, 8], [0, C16], [1, 4]])
                    nc.vector.tensor_mul(Oh, gb, Xh)
                    nc.sync.dma_start(out=od[ci, :, lo:lo + HALF],
                                      in_=O[:, lo:lo + HALF])

            # gate/mul/store runs one chunk behind stats so ACT and DVE
            # stream without waiting on the previous chunk's MLP
            for ci in range(NCHUNK):
                stats(ci)
                if ci:
                    gate_mul_store(ci - 1)
            gate_mul_store(NCHUNK - 1)

    if split_waits:
        _split_multi_waits(nc, mybir)
    return nc


def kernel(x, w1, b1, w2, b2, wv, bv, trace=False):
    global LAST_EXEC_NS
    import ml_dtypes
    from concourse.bass_utils import run_bass_kernel_spmd

    bf = ml_dtypes.bfloat16
    xb = np.asarray(x, np.float32).astype(bf)
    consts = _pack_params(w1, b1, w2, b2, wv, bv)

    nc = _build()

    in_maps = []
    for i in range(NCORES):
        m = {"x": _pack_x(xb[i * BPC:(i + 1) * BPC])}
        m.update(consts)
        in_maps.append(m)

    res = run_bass_kernel_spmd(nc, in_maps, core_ids=list(range(NCORES)),
                               trace=trace)
    LAST_EXEC_NS = res.exec_time_ns

    out = np.empty((B, C, H, W), np.float32)
    for i, r in enumerate(res.results):
        out[i * BPC:(i + 1) * BPC] = _unpack_out(r["out"].astype(np.float32))
    return out


# revision 20
# speedup vs baseline: 1.3938x; 1.3938x over previous
"""Trainium2 Bass kernel for nn_LA_283467842715.

Math (per batch b, head h of 16, each head owning 128 contiguous channels):
  means/maxs over (128 group channels x 2x2 patch) -> [B,16,4,4]
  tiny MLP (16->1 conv, relu, 1->16 conv) on means and maxs, fused by a
  2->1 conv, bilinear-upsampled 4x4->8x8, sigmoid -> gate
  out = x * (1 + gate[b, h, y, x])

v4 (pure data parallel, 32 batches/core, 4 chunks of 8 batches):
  HBM/SBUF layout is HOST-PERMUTED patch-major bf16: partition
  p = b*16+h, free f = ij*512 + c*4 + d, with ij = i*4+j the 4x4 patch
  grid, c the in-group channel and d = dy*2+dx the 2x2 patch offset.
  Every hot access pattern is then dense:
   - ACT: per-patch SUMS are activation(Copy, accum_out) over dense
     512-elem runs (fp32 accumulate), 16/chunk + relu/sigmoid/gate+1.
   - DVE: pairwise bf16 tensor_max tree (dense 2x mode), patch-max
     reduce, two tiny PSUM->SBUF copies, and the final broadcast
     multiply out = gate1 * x (dense innermost run of 4).
   - PE: the whole gate MLP as tiny fp32 matmuls with block-diagonal
     stationaries (K2 columns pre-permuted to patch-major).
   - GpSimd: deliberately UNUSED (concurrent GpSimd + DVE fully blocks
     one of them on the shared SBUF port -- HW-verified).
   - sync: ALL DMA (HWDGE).  Every chunk's loads are issued up front;
     stores follow in FIFO order as each half-multiply lands.
  The gate/multiply/store stage is software-pipelined one chunk behind
  the stats stage so ACT's accum batches and DVE's trees run back to
  back instead of waiting on the previous chunk's MLP.
"""

import sys

if "/opt/trn_rl_repo" not in sys.path:
    sys.path.insert(0, "/opt/trn_rl_repo")

import numpy as np

HEAD = 16
B, C, H, W = 256, 2048, 8, 8
NCORES = 8
BPC = B // NCORES          # 32 batches per core
CHUNK_B = 8                # batches per SBUF chunk (8*16 heads = 128 partitions)
NCHUNK = BPC // CHUNK_B    # 4
C16 = C // HEAD            # 128 channels per head group
SPAT = H * W               # 64
FREE = C16 * SPAT          # 8192 elems per partition

LAST_EXEC_NS = None        # filled when trace=True

# The 16 patch maxes ride a pairwise bf16 TT tree on DVE (the only
# reduction shape with a 2x uop).  The 16 patch sums are split: the
# first N_TREE patches join a small DVE sum tree (~0.42us/patch all
# in), the rest are ACT activation-accumulators (~1.0us/patch, the
# engine's 1-elem/cycle floor).  Measured on HW: DVE tensor_scalar
# accums lower to TENSOR_SCALAR_CACHE_REDUCE at 1x (769ns/patch) so
# trees beat them.
N_TREE = 5



def _upsample_matrix():
    """U[8,4]: bilinear 4->8, half-pixel centers (align_corners=False)."""
    U = np.zeros((8, 4), dtype=np.float64)
    for y in range(8):
        src = (y + 0.5) / 2.0 - 0.5
        i0 = int(np.floor(src))
        t = src - i0
        U[y, min(max(i0, 0), 3)] += 1.0 - t
        U[y, min(max(i0 + 1, 0), 3)] += t
    return U


# pixel index s = y*8+x for patch-major column c' = ij*4 + d
_PERM = np.array([(2 * (ij // 4) + d // 2) * 8 + 2 * (ij % 4) + d % 2
                  for ij in range(16) for d in range(4)])

# consts column layout (fp32, [128, NCONST])
_W1A = 0        # [:, 0:8]    W1a[p=(b,h), b'] = (b==b') * w1[h] / 512
_W1B = 8        # [:, 8:16]   W1b[p=(b,h), b'] = (b==b') * w1[h]
_K2C = 16       # [0:32, 16:80]   K2c[(t,g), c'] = wv[t] * K2[g, s(c')]
_BRW = 80       # [0:8, 80:208]   Brw2[b', (b,h)] = (b==b') * w2[h]
_BETA = 208     # [:, 208]    (wv0+wv1)*b2[h] + bv
_B1 = 209       # [:, 209]    b1
_IDN = 210      # [:, 210:338] identity
NCONST = 338


def _pack_params(w1, b1, w2, b2, wv, bv):
    w1 = np.asarray(w1, np.float64).reshape(HEAD)
    w2 = np.asarray(w2, np.float64).reshape(HEAD)
    b2 = np.asarray(b2, np.float64).reshape(HEAD)
    wv = np.asarray(wv, np.float64).reshape(2)
    bv = float(np.asarray(bv, np.float64))
    b1 = float(np.asarray(b1, np.float64))

    U = _upsample_matrix()
    # K2[g, s] = U[y,i] * U[x,j], g = i*4+j, s = y*8+x
    K2 = np.einsum("yi,xj->ijyx", U, U).reshape(16, 64)
    K2P = K2[:, _PERM]     # columns in patch-major order

    p = np.arange(128)
    CONST = np.zeros((128, NCONST))
    for b in range(CHUNK_B):
        CONST[b * HEAD:(b + 1) * HEAD, _W1A + b] = w1 / 512.0
        CONST[b * HEAD:(b + 1) * HEAD, _W1B + b] = w1
        CONST[b, _BRW + b * HEAD:_BRW + (b + 1) * HEAD] = w2
    CONST[0:16, _K2C:_K2C + 64] = wv[0] * K2P
    CONST[16:32, _K2C:_K2C + 64] = wv[1] * K2P
    CONST[:, _BETA] = (wv[0] + wv[1]) * b2[p % 16] + bv
    CONST[:, _B1] = b1
    CONST[:, _IDN:_IDN + 128] = np.eye(128)
    return {"consts": np.ascontiguousarray(CONST, np.float32)}


def _pack_x(shard):
    """[BPC, C, H, W] (any dtype) -> [NCHUNK, 128, FREE] patch-major."""
    xs = shard.reshape(NCHUNK, CHUNK_B, HEAD, C16, 4, 2, 4, 2)
    #                   ci      b       h     c    i  dy j  dx
    xt = xs.transpose(0, 1, 2, 4, 6, 3, 5, 7)  # ci b h i j c dy dx
    return np.ascontiguousarray(xt).reshape(NCHUNK, 128, FREE)


def _unpack_out(arr):
    """[NCHUNK, 128, FREE] patch-major -> [BPC, C, H, W]."""
    xt = arr.reshape(NCHUNK, CHUNK_B, HEAD, 4, 4, C16, 2, 2)
    xs = xt.transpose(0, 1, 2, 5, 3, 6, 4, 7)  # ci b h c i dy j dx
    return xs.reshape(BPC, C, H, W)


def _split_multi_waits(nc, mybir):
    """Walrus codegen on this path only supports one sync-wait per
    instruction; hoist extras onto standalone InstEventSemaphore waits
    inserted right before, on the same engine."""
    n = 0
    for fn in nc.m.functions:
        for bb in fn.blocks:
            out = []
            for ins in bb.instructions:
                si = getattr(ins, "sync_info", None)
                waits = list(si.on_wait) if (si and si.on_wait) else []
                if len(waits) > 1:
                    for w in waits[:-1]:
                        n += 1
                        ev = mybir.InstEventSemaphore(
                            name=f"WSPLIT-{n}",
                            sync_info=mybir.SyncInfo(on_wait=[w], on_update=[]),
                        )
                        ev.engine = ins.engine
                        out.append(ev)
                    si.on_wait = [waits[-1]]
                out.append(ins)
            bb.instructions[:] = out


def _tree(op, bass, X, T, p0, np_, base):
    """Pairwise reduce over the 128 channels of patches [p0, p0+np_) of
    patch-major X [128, 8192] -> dense [128, np_*4] = [p, (ij, d)] view
    of scratch T at offset `base`.  Dense bf16 TT ops in 2x mode."""
    def ap(view, off, blk, nij, stride):
        v = view[:, :]
        return bass.AP(tensor=v.tensor, offset=v.offset + off,
                       ap=[v.ap[0], [stride, nij], [1, blk]])

    # L1: [p, ij, 512] -> [p, ij, 256]
    op(T[:, base:base + np_ * 256],
       ap(X, p0 * 512, 256, np_, 512), ap(X, p0 * 512 + 256, 256, np_, 512))
    lo, blk, src = base + np_ * 256, 128, base
    while blk >= 4:
        op(T[:, lo:lo + np_ * blk],
           ap(T, src, blk, np_, 2 * blk), ap(T, src + blk, blk, np_, 2 * blk))
        src = lo
        lo += np_ * blk
        blk //= 2
    return T[:, src:src + np_ * 4]


def _build(split_waits=True):
    import concourse.bass as bass
    import concourse.tile as tile
    from concourse import mybir

    f32 = mybir.dt.float32
    bf16 = mybir.dt.bfloat16
    nc = bass.Bass()

    xd = nc.dram_tensor("x", [NCHUNK, 128, FREE], bf16, kind="ExternalInput")
    od = nc.dram_tensor("out", [NCHUNK, 128, FREE], bf16, kind="ExternalOutput")
    cd = nc.dram_tensor("consts", [128, NCONST], f32, kind="ExternalInput")

    AF = mybir.ActivationFunctionType
    HALF = FREE // 2

    with tile.TileContext(nc) as tc:
        with (
            tc.tile_pool(name="singles", bufs=1) as singles,
            tc.tile_pool(name="xin", bufs=NCHUNK) as xpool,
            tc.tile_pool(name="oout", bufs=3) as opool,
            tc.tile_pool(name="small", bufs=2) as small,
            tc.tile_pool(name="psum", bufs=2, space="PSUM") as psum,
        ):
            CN = singles.tile([128, NCONST], f32)
            nc.sync.dma_start(out=CN, in_=cd[:, :])
            w1a = CN[:, _W1A:_W1A + 8]
            w1b = CN[:, _W1B:_W1B + 8]
            k2c = CN[0:32, _K2C:_K2C + 64]
            brw = CN[0:8, _BRW:_BRW + 128]
            beta = CN[:, _BETA:_BETA + 1]
            b1c = CN[0:8, _B1:_B1 + 1]
            idn8 = CN[0:8, _IDN:_IDN + 8]

            TM = singles.tile([128, 8192 + 4096], bf16)  # tree scratch
            DUM = singles.tile([128, 512], bf16)       # ACT accum dummy out
            DUMV = singles.tile([128, 512], bf16)      # DVE accum dummy out
            DUMG = singles.tile([128, 512], bf16)      # GpSimd accum dummy out

            # prefetch every chunk before any store can queue on the ring
            Xs = []
            for ci in range(NCHUNK):
                X = xpool.tile([128, FREE], bf16, tag="X")
                nc.sync.dma_start(out=X[:, 0:HALF], in_=xd[ci, :, 0:HALF])
                nc.sync.dma_start(out=X[:, HALF:FREE], in_=xd[ci, :, HALF:FREE])
                Xs.append(X)

            sms = []

            def stats(ci):
                X = Xs[ci]
                sm = small.tile([128, 32], f32, tag="sm")
                sms.append(sm)
                # patch sums: first N_TREE patches via a DVE pairwise
                # tree, the rest as ACT activation-accumulators.
                ALU = mybir.AluOpType
                for ij in range(N_TREE, 16):
                    nc.scalar.activation(DUM[:, :],
                                         X[:, ij * 512:(ij + 1) * 512],
                                         AF.Copy, accum_out=sm[:, ij:ij + 1])
                s5 = _tree(nc.vector.tensor_add, bass, X, TM, 0, N_TREE, 8192)
                nc.vector.reduce_sum(
                    out=sm[:, 0:N_TREE].rearrange("p (ij o) -> p ij o", o=1),
                    in_=s5.rearrange("p (ij d) -> p ij d", d=4),
                    axis=mybir.AxisListType.X)
                # patch maxes: two DVE trees (one per DMA half)
                mA = _tree(nc.vector.tensor_max, bass, X, TM, 0, 8, 0)
                nc.vector.reduce_max(
                    out=sm[:, 16:24].rearrange("p (ij o) -> p ij o", o=1),
                    in_=mA.rearrange("p (ij d) -> p ij d", d=4),
                    axis=mybir.AxisListType.X)
                mB = _tree(nc.vector.tensor_max, bass, X, TM, 8, 8, 4096)
                nc.vector.reduce_max(
                    out=sm[:, 24:32].rearrange("p (ij o) -> p ij o", o=1),
                    in_=mB.rearrange("p (ij d) -> p ij d", d=4),
                    axis=mybir.AxisListType.X)

            def gate_mul_store(ci):
                X, sm = Xs[ci], sms[ci]
                O = opool.tile([128, FREE], bf16, tag="O")

                P1 = psum.tile([8, 32], f32, tag="P1")
                nc.tensor.matmul(P1[:, 0:16], w1a, sm[:, 0:16])
                nc.tensor.matmul(P1[:, 16:32], w1b, sm[:, 16:32])
                R = small.tile([8, 32], f32, tag="R")
                nc.scalar.activation(R[:, :], P1[:, :], AF.Relu, bias=b1c)

                RT = psum.tile([32, 8], f32, tag="RT")
                nc.tensor.transpose(RT[:, :], R[:, :], idn8)
                RTs = small.tile([32, 8], f32, tag="RTs")
                nc.scalar.copy(RTs[:, :], RT[:, :])

                QP = psum.tile([8, 64], f32, tag="QP")
                nc.tensor.matmul(QP[:, :], RTs[:, :], k2c)
                qup = small.tile([8, 64], f32, tag="qup")
                nc.scalar.copy(qup[:, :], QP[:, :])

                Z = psum.tile([128, 64], f32, tag="Z")
                nc.tensor.matmul(Z[:, :], brw, qup[:, :])
                gate = small.tile([128, 64], bf16, tag="gate")
                nc.scalar.activation(gate[:, :], Z[:, :], AF.Sigmoid, bias=beta)
                gate1 = small.tile([128, 64], bf16, tag="gate1")
                nc.vector.tensor_scalar_add(gate1[:, :], gate[:, :], 1.0)

                # out = gate1 * x (gate broadcast over c), one dense TT
                # per piece so each store trails its piece; the last
                # chunk is cut finer so the final store starts sooner
                g = gate1[:, :]
                npc = 4 if ci == NCHUNK - 1 else 2
                pw = FREE // npc
                for pi in range(npc):
                    lo, nij = pi * pw, 16 // npc
                    Xh = X[:, lo:lo + pw].rearrange(
                        "p (ij c d) -> p ij c d", ij=nij, c=C16)
                    Oh = O[:, lo:lo + pw].rearrange(
                        "p (ij c d) -> p ij c d", ij=nij, c=C16)
                    gb = bass.AP(tensor=g.tensor,
                                 offset=g.offset + pi * nij * 4,
                                 ap=[g.ap[0], [4, nij], [0, C16], [1, 4]])
                    nc.vector.tensor_mul(Oh, gb, Xh)
                    nc.sync.dma_start(out=od[ci, :, lo:lo + pw],
                                      in_=O[:, lo:lo + pw])

            # gate/mul/store runs one chunk behind stats so ACT and DVE
            # stream without waiting on the previous chunk's MLP
            for ci in range(NCHUNK):
                stats(ci)
                if ci:
                    gate_mul_store(ci - 1)
            gate_mul_store(NCHUNK - 1)

    if split_waits:
        _split_multi_waits(nc, mybir)
    return nc


def kernel(x, w1, b1, w2, b2, wv, bv, trace=False):
    global LAST_EXEC_NS
    import ml_dtypes
    from concourse.bass_utils import run_bass_kernel_spmd

    bf = ml_dtypes.bfloat16
    xb = np.asarray(x, np.float32).astype(bf)
    consts = _pack_params(w1, b1, w2, b2, wv, bv)

    nc = _build()

    in_maps = []
    for i in range(NCORES):
        m = {"x": _pack_x(xb[i * BPC:(i + 1) * BPC])}
        m.update(consts)
        in_maps.append(m)

    res = run_bass_kernel_spmd(nc, in_maps, core_ids=list(range(NCORES)),
                               trace=trace)
    LAST_EXEC_NS = res.exec_time_ns

    out = np.empty((B, C, H, W), np.float32)
    for i, r in enumerate(res.results):
        out[i * BPC:(i + 1) * BPC] = _unpack_out(r["out"].astype(np.float32))
    return out


# revision 21
# speedup vs baseline: 1.4688x; 1.0538x over previous
"""Trainium2 Bass kernel for nn_LA_283467842715.

Math (per batch b, head h of 16, each head owning 128 contiguous channels):
  means/maxs over (128 group channels x 2x2 patch) -> [B,16,4,4]
  tiny MLP (16->1 conv, relu, 1->16 conv) on means and maxs, fused by a
  2->1 conv, bilinear-upsampled 4x4->8x8, sigmoid -> gate
  out = x * (1 + gate[b, h, y, x])

v4 (pure data parallel, 32 batches/core, 4 chunks of 8 batches):
  HBM/SBUF layout is HOST-PERMUTED patch-major bf16: partition
  p = b*16+h, free f = ij*512 + c*4 + d, with ij = i*4+j the 4x4 patch
  grid, c the in-group channel and d = dy*2+dx the 2x2 patch offset.
  Every hot access pattern is then dense:
   - ACT: per-patch SUMS are activation(Copy, accum_out) over dense
     512-elem runs (fp32 accumulate), 16/chunk + relu/sigmoid/gate+1.
   - DVE: pairwise bf16 tensor_max tree (dense 2x mode), patch-max
     reduce, two tiny PSUM->SBUF copies, and the final broadcast
     multiply out = gate1 * x (dense innermost run of 4).
   - PE: the whole gate MLP as tiny fp32 matmuls with block-diagonal
     stationaries (K2 columns pre-permuted to patch-major).
   - GpSimd: deliberately UNUSED (concurrent GpSimd + DVE fully blocks
     one of them on the shared SBUF port -- HW-verified).
   - sync: ALL DMA (HWDGE).  Every chunk's loads are issued up front;
     stores follow in FIFO order as each half-multiply lands.
  The gate/multiply/store stage is software-pipelined one chunk behind
  the stats stage so ACT's accum batches and DVE's trees run back to
  back instead of waiting on the previous chunk's MLP.
"""

import sys

if "/opt/trn_rl_repo" not in sys.path:
    sys.path.insert(0, "/opt/trn_rl_repo")

import numpy as np

HEAD = 16
B, C, H, W = 256, 2048, 8, 8
NCORES = 8
BPC = B // NCORES          # 32 batches per core
CHUNK_B = 8                # batches per SBUF chunk (8*16 heads = 128 partitions)
NCHUNK = BPC // CHUNK_B    # 4
C16 = C // HEAD            # 128 channels per head group
SPAT = H * W               # 64
FREE = C16 * SPAT          # 8192 elems per partition

LAST_EXEC_NS = None        # filled when trace=True

# The 16 patch maxes ride a pairwise bf16 TT tree on DVE (the only
# reduction shape with a 2x uop).  The 16 patch sums are split: the
# first N_TREE patches join a small DVE sum tree (~0.42us/patch all
# in), the rest are ACT activation-accumulators (~1.0us/patch, the
# engine's 1-elem/cycle floor).  Measured on HW: DVE tensor_scalar
# accums lower to TENSOR_SCALAR_CACHE_REDUCE at 1x (769ns/patch) so
# trees beat them.
N_TREE = 5



def _upsample_matrix():
    """U[8,4]: bilinear 4->8, half-pixel centers (align_corners=False)."""
    U = np.zeros((8, 4), dtype=np.float64)
    for y in range(8):
        src = (y + 0.5) / 2.0 - 0.5
        i0 = int(np.floor(src))
        t = src - i0
        U[y, min(max(i0, 0), 3)] += 1.0 - t
        U[y, min(max(i0 + 1, 0), 3)] += t
    return U


# pixel index s = y*8+x for patch-major column c' = ij*4 + d
_PERM = np.array([(2 * (ij // 4) + d // 2) * 8 + 2 * (ij % 4) + d % 2
                  for ij in range(16) for d in range(4)])

# consts column layout (fp32, [128, NCONST])
_W1A = 0        # [:, 0:8]    W1a[p=(b,h), b'] = (b==b') * w1[h] / 512
_W1B = 8        # [:, 8:16]   W1b[p=(b,h), b'] = (b==b') * w1[h]
_K2C = 16       # [0:32, 16:80]   K2c[(t,g), c'] = wv[t] * K2[g, s(c')]
_BRW = 80       # [0:8, 80:208]   Brw2[b', (b,h)] = (b==b') * w2[h]
_BETA = 208     # [:, 208]    (wv0+wv1)*b2[h] + bv
_B1 = 209       # [:, 209]    b1
_IDN = 210      # [:, 210:338] identity
NCONST = 338


def _pack_params(w1, b1, w2, b2, wv, bv):
    w1 = np.asarray(w1, np.float64).reshape(HEAD)
    w2 = np.asarray(w2, np.float64).reshape(HEAD)
    b2 = np.asarray(b2, np.float64).reshape(HEAD)
    wv = np.asarray(wv, np.float64).reshape(2)
    bv = float(np.asarray(bv, np.float64))
    b1 = float(np.asarray(b1, np.float64))

    U = _upsample_matrix()
    # K2[g, s] = U[y,i] * U[x,j], g = i*4+j, s = y*8+x
    K2 = np.einsum("yi,xj->ijyx", U, U).reshape(16, 64)
    K2P = K2[:, _PERM]     # columns in patch-major order

    p = np.arange(128)
    CONST = np.zeros((128, NCONST))
    for b in range(CHUNK_B):
        CONST[b * HEAD:(b + 1) * HEAD, _W1A + b] = w1 / 512.0
        CONST[b * HEAD:(b + 1) * HEAD, _W1B + b] = w1
        CONST[b, _BRW + b * HEAD:_BRW + (b + 1) * HEAD] = w2
    CONST[0:16, _K2C:_K2C + 64] = wv[0] * K2P
    CONST[16:32, _K2C:_K2C + 64] = wv[1] * K2P
    CONST[:, _BETA] = (wv[0] + wv[1]) * b2[p % 16] + bv
    CONST[:, _B1] = b1
    CONST[:, _IDN:_IDN + 128] = np.eye(128)
    return {"consts": np.ascontiguousarray(CONST, np.float32)}


def _pack_x(shard):
    """[BPC, C, H, W] (any dtype) -> [NCHUNK, 128, FREE] patch-major."""
    xs = shard.reshape(NCHUNK, CHUNK_B, HEAD, C16, 4, 2, 4, 2)
    #                   ci      b       h     c    i  dy j  dx
    xt = xs.transpose(0, 1, 2, 4, 6, 3, 5, 7)  # ci b h i j c dy dx
    return np.ascontiguousarray(xt).reshape(NCHUNK, 128, FREE)


def _unpack_out(arr):
    """[NCHUNK, 128, FREE] patch-major -> [BPC, C, H, W]."""
    xt = arr.reshape(NCHUNK, CHUNK_B, HEAD, 4, 4, C16, 2, 2)
    xs = xt.transpose(0, 1, 2, 5, 3, 6, 4, 7)  # ci b h c i dy j dx
    return xs.reshape(BPC, C, H, W)


def _split_multi_waits(nc, mybir):
    """Walrus codegen on this path only supports one sync-wait per
    instruction; hoist extras onto standalone InstEventSemaphore waits
    inserted right before, on the same engine."""
    n = 0
    for fn in nc.m.functions:
        for bb in fn.blocks:
            out = []
            for ins in bb.instructions:
                si = getattr(ins, "sync_info", None)
                waits = list(si.on_wait) if (si and si.on_wait) else []
                if len(waits) > 1:
                    for w in waits[:-1]:
                        n += 1
                        ev = mybir.InstEventSemaphore(
                            name=f"WSPLIT-{n}",
                            sync_info=mybir.SyncInfo(on_wait=[w], on_update=[]),
                        )
                        ev.engine = ins.engine
                        out.append(ev)
                    si.on_wait = [waits[-1]]
                out.append(ins)
            bb.instructions[:] = out


def _tree(op, bass, X, T, p0, np_, base):
    """Pairwise reduce over the 128 channels of patches [p0, p0+np_) of
    patch-major X [128, 8192] -> dense [128, np_*4] = [p, (ij, d)] view
    of scratch T at offset `base`.  Dense bf16 TT ops in 2x mode."""
    def ap(view, off, blk, nij, stride):
        v = view[:, :]
        return bass.AP(tensor=v.tensor, offset=v.offset + off,
                       ap=[v.ap[0], [stride, nij], [1, blk]])

    # L1: [p, ij, 512] -> [p, ij, 256]
    op(T[:, base:base + np_ * 256],
       ap(X, p0 * 512, 256, np_, 512), ap(X, p0 * 512 + 256, 256, np_, 512))
    lo, blk, src = base + np_ * 256, 128, base
    while blk >= 4:
        op(T[:, lo:lo + np_ * blk],
           ap(T, src, blk, np_, 2 * blk), ap(T, src + blk, blk, np_, 2 * blk))
        src = lo
        lo += np_ * blk
        blk //= 2
    return T[:, src:src + np_ * 4]


def _build(split_waits=True):
    import concourse.bass as bass
    import concourse.tile as tile
    from concourse import mybir

    f32 = mybir.dt.float32
    bf16 = mybir.dt.bfloat16
    nc = bass.Bass()

    xd = nc.dram_tensor("x", [NCHUNK, 128, FREE], bf16, kind="ExternalInput")
    od = nc.dram_tensor("out", [NCHUNK, 128, FREE], bf16, kind="ExternalOutput")
    cd = nc.dram_tensor("consts", [128, NCONST], f32, kind="ExternalInput")

    AF = mybir.ActivationFunctionType
    HALF = FREE // 2

    with tile.TileContext(nc) as tc:
        with (
            tc.tile_pool(name="singles", bufs=1) as singles,
            tc.tile_pool(name="xin", bufs=NCHUNK) as xpool,
            tc.tile_pool(name="oout", bufs=3) as opool,
            tc.tile_pool(name="small", bufs=2) as small,
            tc.tile_pool(name="psum", bufs=2, space="PSUM") as psum,
        ):
            CN = singles.tile([128, NCONST], f32)
            nc.sync.dma_start(out=CN, in_=cd[:, :])
            w1a = CN[:, _W1A:_W1A + 8]
            w1b = CN[:, _W1B:_W1B + 8]
            k2c = CN[0:32, _K2C:_K2C + 64]
            brw = CN[0:8, _BRW:_BRW + 128]
            beta = CN[:, _BETA:_BETA + 1]
            b1c = CN[0:8, _B1:_B1 + 1]
            idn8 = CN[0:8, _IDN:_IDN + 8]

            TM = singles.tile([128, 8192 + 4096], bf16)  # tree scratch
            DUM = singles.tile([128, 512], bf16)       # ACT accum dummy out
            DUMV = singles.tile([128, 512], bf16)      # DVE accum dummy out
            DUMG = singles.tile([128, 512], bf16)      # GpSimd accum dummy out

            # prefetch every chunk before any store can queue on the ring
            Xs = []
            for ci in range(NCHUNK):
                X = xpool.tile([128, FREE], bf16, tag="X")
                nc.sync.dma_start(out=X[:, 0:HALF], in_=xd[ci, :, 0:HALF])
                nc.sync.dma_start(out=X[:, HALF:FREE], in_=xd[ci, :, HALF:FREE])
                Xs.append(X)

            sms = []

            def stats(ci):
                X = Xs[ci]
                sm = small.tile([128, 32], f32, tag="sm")
                sms.append(sm)
                # patch sums: first N_TREE patches via a DVE pairwise
                # tree, the rest as ACT activation-accumulators.
                ALU = mybir.AluOpType
                for ij in range(N_TREE, 16):
                    nc.scalar.activation(DUM[:, :],
                                         X[:, ij * 512:(ij + 1) * 512],
                                         AF.Copy, accum_out=sm[:, ij:ij + 1])
                s5 = _tree(nc.vector.tensor_add, bass, X, TM, 0, N_TREE, 8192)
                nc.vector.reduce_sum(
                    out=sm[:, 0:N_TREE].rearrange("p (ij o) -> p ij o", o=1),
                    in_=s5.rearrange("p (ij d) -> p ij d", d=4),
                    axis=mybir.AxisListType.X)
                # patch maxes: two DVE trees (one per DMA half)
                mA = _tree(nc.vector.tensor_max, bass, X, TM, 0, 8, 0)
                nc.vector.reduce_max(
                    out=sm[:, 16:24].rearrange("p (ij o) -> p ij o", o=1),
                    in_=mA.rearrange("p (ij d) -> p ij d", d=4),
                    axis=mybir.AxisListType.X)
                mB = _tree(nc.vector.tensor_max, bass, X, TM, 8, 8, 4096)
                nc.vector.reduce_max(
                    out=sm[:, 24:32].rearrange("p (ij o) -> p ij o", o=1),
                    in_=mB.rearrange("p (ij d) -> p ij d", d=4),
                    axis=mybir.AxisListType.X)

            def gate_mul_store(ci):
                X, sm = Xs[ci], sms[ci]
                O = opool.tile([128, FREE], bf16, tag="O")

                P1 = psum.tile([8, 32], f32, tag="P1")
                nc.tensor.matmul(P1[:, 0:16], w1a, sm[:, 0:16])
                nc.tensor.matmul(P1[:, 16:32], w1b, sm[:, 16:32])
                R = small.tile([8, 32], f32, tag="R")
                nc.scalar.activation(R[:, :], P1[:, :], AF.Relu, bias=b1c)

                RT = psum.tile([32, 8], f32, tag="RT")
                nc.tensor.transpose(RT[:, :], R[:, :], idn8)
                RTs = small.tile([32, 8], f32, tag="RTs")
                nc.scalar.copy(RTs[:, :], RT[:, :])

                QP = psum.tile([8, 64], f32, tag="QP")
                nc.tensor.matmul(QP[:, :], RTs[:, :], k2c)
                qup = small.tile([8, 64], f32, tag="qup")
                nc.scalar.copy(qup[:, :], QP[:, :])

                Z = psum.tile([128, 64], f32, tag="Z")
                nc.tensor.matmul(Z[:, :], brw, qup[:, :])
                gate = small.tile([128, 64], bf16, tag="gate")
                nc.scalar.activation(gate[:, :], Z[:, :], AF.Sigmoid, bias=beta)
                gate1 = small.tile([128, 64], bf16, tag="gate1")
                nc.scalar.activation(gate1[:, :], gate[:, :], AF.Copy, bias=1.0)

                # out = gate1 * x (gate broadcast over c), one dense TT
                # per piece so each store trails its piece; the last
                # chunk is cut finer so the final store starts sooner
                g = gate1[:, :]
                npc = 4 if ci == NCHUNK - 1 else 2
                pw = FREE // npc
                for pi in range(npc):
                    lo, nij = pi * pw, 16 // npc
                    Xh = X[:, lo:lo + pw].rearrange(
                        "p (ij c d) -> p ij c d", ij=nij, c=C16)
                    Oh = O[:, lo:lo + pw].rearrange(
                        "p (ij c d) -> p ij c d", ij=nij, c=C16)
                    gb = bass.AP(tensor=g.tensor,
                                 offset=g.offset + pi * nij * 4,
                                 ap=[g.ap[0], [4, nij], [0, C16], [1, 4]])
                    nc.vector.tensor_mul(Oh, gb, Xh)
                    nc.sync.dma_start(out=od[ci, :, lo:lo + pw],
                                      in_=O[:, lo:lo + pw])

            # gate/mul/store runs one chunk behind stats so ACT and DVE
            # stream without waiting on the previous chunk's MLP
            for ci in range(NCHUNK):
                stats(ci)
                if ci:
                    gate_mul_store(ci - 1)
            gate_mul_store(NCHUNK - 1)

    if split_waits:
        _split_multi_waits(nc, mybir)
    return nc


def kernel(x, w1, b1, w2, b2, wv, bv, trace=False):
    global LAST_EXEC_NS
    import ml_dtypes
    from concourse.bass_utils import run_bass_kernel_spmd

    bf = ml_dtypes.bfloat16
    xb = np.asarray(x, np.float32).astype(bf)
    consts = _pack_params(w1, b1, w2, b2, wv, bv)

    nc = _build()

    in_maps = []
    for i in range(NCORES):
        m = {"x": _pack_x(xb[i * BPC:(i + 1) * BPC])}
        m.update(consts)
        in_maps.append(m)

    res = run_bass_kernel_spmd(nc, in_maps, core_ids=list(range(NCORES)),
                               trace=trace)
    LAST_EXEC_NS = res.exec_time_ns

    out = np.empty((B, C, H, W), np.float32)
    for i, r in enumerate(res.results):
        out[i * BPC:(i + 1) * BPC] = _unpack_out(r["out"].astype(np.float32))
    return out
